# revision 4
# baseline (speedup 1.0000x reference)
"""Pairwise-distance loss kernel for Trainium2 (8 NeuronCores, SPMD).

loss = (total_sum - 2*diag_sum) / B * 0.1  over the [B, B] matrix
d[i, n] = ||output[i] - target[n]||_2,  B=8192, D=128.

Sharding: core c owns rows [c*1024, (c+1)*1024) of `output` and all 8192
`target` columns. Per 128-row block m and 2048-col group g (4 PSUM banks):
  PSUM[i, n] = yy[n] - 2 * x_i . y_n   via ONE fp8 DoubleRow matmul per
  512-col bank: K=256 packed as [128, 2]: plane 0 carries the 128 features
  (weights = xq, moving = -2*yq), plane 1 carries the rank-1 yy term
  (weights row0 = 4.0, moving row0 = yy/4 in fp8).
  ACT: d = sqrt(PSUM + xx_i) with per-partition bias, accum_out = row sums.
Diagonal: each core's target columns are rotated by -c*1024 on the host so
the diagonal of row-block m always sits at local columns [m*128, (m+1)*128)
of group 0 — extracted with an eye-mask multiply + reduce on the vector
engine. The 8 cores' partial sums are combined on the host.
"""

import numpy as np
import ml_dtypes
from contextlib import ExitStack

B = 8192
D = 128
C = 8          # cores
M = B // C     # 1024 rows per core
P = 128        # partitions / row-block height
NM = M // P    # 8 row-blocks per core
GW = 2048      # ACT group width (4 PSUM banks)
NG = B // GW   # 4 groups
TS = 512       # matmul moving-dim tile (1 PSUM bank of f32)
NS = GW // TS  # 4 slices per group

_F8 = np.dtype(ml_dtypes.float8_e4m3)

# test.py can flip these before calling kernel() to capture an NTFF profile.
TRACE = False
LAST_RESULT = None

_nc = None


def _build():
    from concourse import bacc, bass, tile, mybir

    f32 = mybir.dt.float32
    fp8 = mybir.dt.float8e4
    nc = bacc.Bacc("TRN2", target_bir_lowering=False, debug=False)

    w8 = nc.dram_tensor("w8", [P, NM, 2, P], fp8, kind="ExternalInput").ap()
    rhs8 = nc.dram_tensor("rhs8", [P, 2, B], fp8, kind="ExternalInput").ap()
    xxT = nc.dram_tensor("xxT", [P, NM], f32, kind="ExternalInput").ap()
    eye = nc.dram_tensor("eye", [P, P], f32, kind="ExternalInput").ap()
    out = nc.dram_tensor("out", [P, NM * NG + NM], f32, kind="ExternalOutput").ap()

    with tile.TileContext(nc) as tc, ExitStack() as ctx:
        const = ctx.enter_context(tc.tile_pool(name="const", bufs=1))
        psum = ctx.enter_context(
            tc.tile_pool(name="psum", bufs=2, space=bass.MemorySpace.PSUM)
        )
        dpool = ctx.enter_context(tc.tile_pool(name="dtile", bufs=3))

        xx_s = const.tile([P, NM], f32)
        nc.sync.dma_start(xx_s[:], xxT[:])
        w8_s = const.tile([P, NM, 2, P], fp8)
        nc.sync.dma_start(w8_s[:], w8[:])
        eye_s = const.tile([P, P], f32)
        nc.sync.dma_start(eye_s[:], eye[:])
        rhs_s = const.tile([P, 2, B], fp8)
        # chunked so group g's matmuls only wait on chunk g, and serialized
        # so chunk 0 gets full DMA bandwidth and lands first
        from concourse.tile_rust import add_dep_helper

        prev = None
        for g in range(NG):
            ins = nc.sync.dma_start(
                rhs_s[:, :, g * GW : (g + 1) * GW],
                rhs8[:, :, g * GW : (g + 1) * GW],
            )
            if prev is not None:
                add_dep_helper(
                    ins.ins, prev.ins, sync=True, reason="serialize rhs chunks"
                )
            prev = ins

        accT = const.tile([P, NM * NG], f32)
        accD = const.tile([P, NM], f32)
        scr = const.tile([P, P], f32)

        for m in range(NM):
            for g in range(NG):
                pt = psum.tile([P, GW], f32)
                for s in range(NS):
                    n0 = g * GW + s * TS
                    nc.tensor.matmul(
                        pt[:, s * TS : (s + 1) * TS],
                        w8_s[:, m],
                        rhs_s[:, :, n0 : n0 + TS],
                        start=True,
                        stop=True,
                        perf_mode=mybir.MatmulPerfMode.DoubleRow,
                    )
                dt_ = dpool.tile([P, GW], f32)
                nc.scalar.activation(
                    dt_[:],
                    pt[:],
                    mybir.ActivationFunctionType.Sqrt,
                    bias=xx_s[:, m : m + 1],
                    scale=1.0,
                    accum_out=accT[:, m * NG + g : m * NG + g + 1],
                )
                if g == 0:
                    # diagonal of this row-block lives at local cols
                    # [m*128, (m+1)*128) thanks to the host-side rotation
                    # (tensor_tensor_reduce is avoided: it wedges the HW)
                    nc.vector.tensor_tensor(
                        out=scr[:],
                        in0=dt_[:, m * P : (m + 1) * P],
                        in1=eye_s[:],
                        op=mybir.AluOpType.mult,
                    )
                    nc.vector.reduce_sum(
                        accD[:, m : m + 1], scr[:], axis=mybir.AxisListType.X
                    )

        nc.sync.dma_start(out[:, 0 : NM * NG], accT[:])
        nc.sync.dma_start(out[:, NM * NG : NM * NG + NM], accD[:])

    nc.compile()
    return nc


def _in_maps(output, target):
    x = np.asarray(output, dtype=np.float32)
    y = np.asarray(target, dtype=np.float32)
    xq = x.astype(_F8)          # [B, D] fp8
    yq = y.astype(_F8)
    xqf = xq.astype(np.float32)
    yqf = yq.astype(np.float32)
    xx = np.einsum("ij,ij->i", xqf, xqf)             # [B] f32
    yy = np.einsum("ij,ij->i", yqf, yqf)             # [B] f32
    m2yqT = np.ascontiguousarray((-2.0 * yqf).T.astype(_F8))  # [D, B], exact
    yy4 = (yy / 4.0).astype(_F8)                     # [B] fp8
    eye = np.eye(P, dtype=np.float32)
    four = np.float32(4.0).astype(_F8)

    maps = []
    for c in range(C):
        rows = slice(c * M, (c + 1) * M)
        w8 = np.zeros((P, NM, 2, P), _F8)
        w8[:, :, 0, :] = xq[rows].T.reshape(P, NM, P)
        w8[0, :, 1, :] = four
        rhs8 = np.zeros((P, 2, B), _F8)
        rhs8[:, 0, :] = np.roll(m2yqT, -c * M, axis=1)
        rhs8[0, 1, :] = np.roll(yy4, -c * M)
        maps.append(
            {
                "w8": w8,
                "rhs8": rhs8,
                "xxT": np.ascontiguousarray(xx[rows].reshape(NM, P).T),
                "eye": eye,
            }
        )
    return maps


def kernel(output, target):
    global _nc, LAST_RESULT
    if _nc is None:
        _nc = _build()
    maps = _in_maps(output, target)

    from concourse.bass_utils import run_bass_kernel_spmd

    res = run_bass_kernel_spmd(
        _nc, maps, core_ids=list(range(C)), trace=TRACE
    )
    LAST_RESULT = res

    tot = np.float64(0.0)
    dg = np.float64(0.0)
    for r in res.results:
        o = np.asarray(r["out"], dtype=np.float64)
        tot += o[:, : NM * NG].sum()
        dg += o[:, NM * NG : NM * NG + NM].sum()
    loss = (tot - 2.0 * dg) / B * 0.1
    return np.float32(loss)


# revision 7
# speedup vs baseline: 1.0575x; 1.0575x over previous
"""Pairwise-distance loss kernel for Trainium2 (8 NeuronCores, SPMD).

loss = (total_sum - 2*diag_sum) / B * 0.1  over the [B, B] matrix
d[i, n] = ||output[i] - target[n]||_2,  B=8192, D=128.

Sharding: core c owns rows [c*1024, (c+1)*1024) of `output` and all 8192
`target` columns. Per 128-row block m and 2048-col group g (4 PSUM banks):
  PSUM[i, n] = yy[n] - 2 * x_i . y_n   via ONE fp8 DoubleRow matmul per
  512-col bank: K=256 packed as [128, 2]: plane 0 carries the 128 features
  (weights = xq, moving = -2*yq), plane 1 carries the rank-1 yy term
  (weights row0 = 4.0, moving row0 = yy/4 in fp8).
  ACT: d = sqrt(PSUM + xx_i) with per-partition bias, accum_out = row sums.
Diagonal: each core's target columns are rotated by -c*1024 on the host so
the diagonal of row-block m always sits at local columns [m*128, (m+1)*128)
of group 0 — extracted with an eye-mask multiply + reduce on the vector
engine. The 8 cores' partial sums are combined on the host.
"""

import numpy as np
import ml_dtypes
from contextlib import ExitStack

B = 8192
D = 128
C = 8          # cores
M = B // C     # 1024 rows per core
P = 128        # partitions / row-block height
NM = M // P    # 8 row-blocks per core
GW = 2048      # ACT group width (4 PSUM banks)
NG = B // GW   # 4 groups
TS = 512       # matmul moving-dim tile (1 PSUM bank of f32)
NS = GW // TS  # 4 slices per group

_F8 = np.dtype(ml_dtypes.float8_e4m3)

# test.py can flip these before calling kernel() to capture an NTFF profile.
TRACE = False
LAST_RESULT = None

_nc = None


def _build():
    from concourse import bacc, bass, tile, mybir

    f32 = mybir.dt.float32
    fp8 = mybir.dt.float8e4
    nc = bacc.Bacc("TRN2", target_bir_lowering=False, debug=False)

    w8 = nc.dram_tensor("w8", [P, NM, 2, P], fp8, kind="ExternalInput").ap()
    rhs8m = nc.dram_tensor("rhs8m", [P, B], fp8, kind="ExternalInput").ap()
    rhs8yy = nc.dram_tensor("rhs8yy", [1, B], fp8, kind="ExternalInput").ap()
    xxT = nc.dram_tensor("xxT", [P, NM], f32, kind="ExternalInput").ap()
    eye = nc.dram_tensor("eye", [P, P], f32, kind="ExternalInput").ap()
    out = nc.dram_tensor("out", [P, NM * NG + NM], f32, kind="ExternalOutput").ap()

    with tile.TileContext(nc) as tc, ExitStack() as ctx:
        const = ctx.enter_context(tc.tile_pool(name="const", bufs=1))
        psum = ctx.enter_context(
            tc.tile_pool(name="psum", bufs=2, space=bass.MemorySpace.PSUM)
        )
        dpool = ctx.enter_context(tc.tile_pool(name="dtile", bufs=3))

        xx_s = const.tile([P, NM], f32)
        nc.sync.dma_start(xx_s[:], xxT[:])
        w8_s = const.tile([P, NM, 2, P], fp8)
        nc.sync.dma_start(w8_s[:], w8[:])
        eye_s = const.tile([P, P], f32)
        nc.sync.dma_start(eye_s[:], eye[:])
        rhs_s = const.tile([P, 2, B], fp8)
        # plane 0 (features) comes from DRAM, chunked so group g's matmuls
        # only wait on chunk g; plane 1 rows 1.. are zeroed on idle engines
        # (their products hit zero weights, but must not be NaN), row 0 = yy/4
        for g in range(NG):
            nc.sync.dma_start(
                rhs_s[:, 0, g * GW : (g + 1) * GW],
                rhs8m[:, g * GW : (g + 1) * GW],
            )
        nc.vector.memset(rhs_s[:, 1, 0 : B // 2], 0.0)
        nc.gpsimd.memset(rhs_s[:, 1, B // 2 : B], 0.0)
        nc.sync.dma_start(rhs_s[0:1, 1, :], rhs8yy[:, :])

        accT = const.tile([P, NM * NG], f32)
        accD = const.tile([P, NM], f32)
        scr = const.tile([P, P], f32)

        for m in range(NM):
            for g in range(NG):
                pt = psum.tile([P, GW], f32)
                for s in range(NS):
                    n0 = g * GW + s * TS
                    nc.tensor.matmul(
                        pt[:, s * TS : (s + 1) * TS],
                        w8_s[:, m],
                        rhs_s[:, :, n0 : n0 + TS],
                        start=True,
                        stop=True,
                        perf_mode=mybir.MatmulPerfMode.DoubleRow,
                    )
                dt_ = dpool.tile([P, GW], f32)
                nc.scalar.activation(
                    dt_[:],
                    pt[:],
                    mybir.ActivationFunctionType.Sqrt,
                    bias=xx_s[:, m : m + 1],
                    scale=1.0,
                    accum_out=accT[:, m * NG + g : m * NG + g + 1],
                )
                if g == 0:
                    # diagonal of this row-block lives at local cols
                    # [m*128, (m+1)*128) thanks to the host-side rotation
                    # (tensor_tensor_reduce is avoided: it wedges the HW)
                    nc.vector.tensor_tensor(
                        out=scr[:],
                        in0=dt_[:, m * P : (m + 1) * P],
                        in1=eye_s[:],
                        op=mybir.AluOpType.mult,
                    )
                    nc.vector.reduce_sum(
                        accD[:, m : m + 1], scr[:], axis=mybir.AxisListType.X
                    )

        nc.sync.dma_start(out[:, 0 : NM * NG], accT[:])
        nc.sync.dma_start(out[:, NM * NG : NM * NG + NM], accD[:])

    nc.compile()
    return nc


def _in_maps(output, target):
    x = np.asarray(output, dtype=np.float32)
    y = np.asarray(target, dtype=np.float32)
    xq = x.astype(_F8)          # [B, D] fp8
    yq = y.astype(_F8)
    xqf = xq.astype(np.float32)
    yqf = yq.astype(np.float32)
    xx = np.einsum("ij,ij->i", xqf, xqf)             # [B] f32
    yy = np.einsum("ij,ij->i", yqf, yqf)             # [B] f32
    m2yqT = np.ascontiguousarray((-2.0 * yqf).T.astype(_F8))  # [D, B], exact
    yy4 = (yy / 4.0).astype(_F8)                     # [B] fp8
    eye = np.eye(P, dtype=np.float32)
    four = np.float32(4.0).astype(_F8)

    maps = []
    for c in range(C):
        rows = slice(c * M, (c + 1) * M)
        w8 = np.zeros((P, NM, 2, P), _F8)
        w8[:, :, 0, :] = xq[rows].T.reshape(P, NM, P)
        w8[0, :, 1, :] = four
        maps.append(
            {
                "w8": w8,
                "rhs8m": np.ascontiguousarray(np.roll(m2yqT, -c * M, axis=1)),
                "rhs8yy": np.ascontiguousarray(np.roll(yy4, -c * M)[None, :]),
                "xxT": np.ascontiguousarray(xx[rows].reshape(NM, P).T),
                "eye": eye,
            }
        )
    return maps


def kernel(output, target):
    global _nc, LAST_RESULT
    if _nc is None:
        _nc = _build()
    maps = _in_maps(output, target)

    from concourse.bass_utils import run_bass_kernel_spmd

    res = run_bass_kernel_spmd(
        _nc, maps, core_ids=list(range(C)), trace=TRACE
    )
    LAST_RESULT = res

    tot = np.float64(0.0)
    dg = np.float64(0.0)
    for r in res.results:
        o = np.asarray(r["out"], dtype=np.float64)
        tot += o[:, : NM * NG].sum()
        dg += o[:, NM * NG : NM * NG + NM].sum()
    loss = (tot - 2.0 * dg) / B * 0.1
    return np.float32(loss)


# revision 8
# speedup vs baseline: 1.0801x; 1.0214x over previous
"""Pairwise-distance loss kernel for Trainium2 (8 NeuronCores, SPMD).

loss = (total_sum - 2*diag_sum) / B * 0.1  over the [B, B] matrix
d[i, n] = ||output[i] - target[n]||_2,  B=8192, D=128.

Sharding: core c owns rows [c*1024, (c+1)*1024) of `output` and all 8192
`target` columns. Per 128-row block m and 2048-col group g (4 PSUM banks):
  PSUM[i, n] = yy[n] - 2 * x_i . y_n   via ONE fp8 DoubleRow matmul per
  512-col bank: K=256 packed as [128, 2]: plane 0 carries the 128 features
  (weights = xq, moving = -2*yq), plane 1 carries the rank-1 yy term
  (weights row0 = 4.0, moving row0 = yy/4 in fp8).
  ACT: d = sqrt(PSUM + xx_i) with per-partition bias, accum_out = row sums.
Diagonal: each core's target columns are rotated by -c*1024 on the host so
the diagonal of row-block m always sits at local columns [m*128, (m+1)*128)
of group 0 — extracted with an eye-mask multiply + reduce on the vector
engine. The 8 cores' partial sums are combined on the host.
"""

import numpy as np
import ml_dtypes
from contextlib import ExitStack

B = 8192
D = 128
C = 8          # cores
M = B // C     # 1024 rows per core
P = 128        # partitions / row-block height
NM = M // P    # 8 row-blocks per core
GW = 2048      # ACT group width (4 PSUM banks)
NG = B // GW   # 4 groups
TS = 512       # matmul moving-dim tile (1 PSUM bank of f32)
NS = GW // TS  # 4 slices per group

_F8 = np.dtype(ml_dtypes.float8_e4m3)

# test.py can flip these before calling kernel() to capture an NTFF profile.
TRACE = False
LAST_RESULT = None

_nc = None


def _build():
    from concourse import bacc, bass, tile, mybir

    f32 = mybir.dt.float32
    fp8 = mybir.dt.float8e4
    nc = bacc.Bacc("TRN2", target_bir_lowering=False, debug=False)

    w8 = nc.dram_tensor("w8", [P, NM, 2, P], fp8, kind="ExternalInput").ap()
    rhs8m = nc.dram_tensor("rhs8m", [P, B], fp8, kind="ExternalInput").ap()
    rhs8yy = nc.dram_tensor("rhs8yy", [1, B], fp8, kind="ExternalInput").ap()
    xxT = nc.dram_tensor("xxT", [P, NM], f32, kind="ExternalInput").ap()
    eye = nc.dram_tensor("eye", [P, P], f32, kind="ExternalInput").ap()
    out = nc.dram_tensor("out", [P, NM * NG + NM], f32, kind="ExternalOutput").ap()

    with tile.TileContext(nc) as tc, ExitStack() as ctx:
        const = ctx.enter_context(tc.tile_pool(name="const", bufs=1))
        psum = ctx.enter_context(
            tc.tile_pool(name="psum", bufs=2, space=bass.MemorySpace.PSUM)
        )
        dpool = ctx.enter_context(tc.tile_pool(name="dtile", bufs=3))

        xx_s = const.tile([P, NM], f32)
        nc.sync.dma_start(xx_s[:], xxT[:])
        w8_s = const.tile([P, NM, 2, P], fp8)
        nc.sync.dma_start(w8_s[:], w8[:])
        eye_s = const.tile([P, P], f32)
        nc.sync.dma_start(eye_s[:], eye[:])
        rhs_s = const.tile([P, 2, B], fp8)
        # plane 0 (features) comes from DRAM, chunked so group g's matmuls
        # only wait on chunk g; plane 1 rows 1.. are zeroed on idle engines
        # (their products hit zero weights, but must not be NaN), row 0 = yy/4
        for g in range(NG):
            nc.sync.dma_start(
                rhs_s[:, 0, g * GW : (g + 1) * GW],
                rhs8m[:, g * GW : (g + 1) * GW],
            )
        plane1_u32 = rhs_s[:, 1, :].bitcast(mybir.dt.uint32)  # [P, B//4]
        nc.vector.memset(plane1_u32[:, 0 : B // 8], 0)
        nc.gpsimd.memset(plane1_u32[:, B // 8 : B // 4], 0)
        # separate DMA ring (gpsimd SWDGE) so this doesn't queue behind the
        # bulk feature chunks on the sync ring
        nc.gpsimd.dma_start(rhs_s[0:1, 1, :], rhs8yy[:, :])

        accT = const.tile([P, NM * NG], f32)
        accD = const.tile([P, NM], f32)
        scr = const.tile([P, P], f32)

        for m in range(NM):
            for g in range(NG):
                pt = psum.tile([P, GW], f32)
                for s in range(NS):
                    n0 = g * GW + s * TS
                    nc.tensor.matmul(
                        pt[:, s * TS : (s + 1) * TS],
                        w8_s[:, m],
                        rhs_s[:, :, n0 : n0 + TS],
                        start=True,
                        stop=True,
                        perf_mode=mybir.MatmulPerfMode.DoubleRow,
                    )
                dt_ = dpool.tile([P, GW], f32)
                nc.scalar.activation(
                    dt_[:],
                    pt[:],
                    mybir.ActivationFunctionType.Sqrt,
                    bias=xx_s[:, m : m + 1],
                    scale=1.0,
                    accum_out=accT[:, m * NG + g : m * NG + g + 1],
                )
                if g == 0:
                    # diagonal of this row-block lives at local cols
                    # [m*128, (m+1)*128) thanks to the host-side rotation
                    # (tensor_tensor_reduce is avoided: it wedges the HW)
                    nc.vector.tensor_tensor(
                        out=scr[:],
                        in0=dt_[:, m * P : (m + 1) * P],
                        in1=eye_s[:],
                        op=mybir.AluOpType.mult,
                    )
                    nc.vector.reduce_sum(
                        accD[:, m : m + 1], scr[:], axis=mybir.AxisListType.X
                    )

        nc.sync.dma_start(out[:, 0 : NM * NG], accT[:])
        nc.sync.dma_start(out[:, NM * NG : NM * NG + NM], accD[:])

    nc.compile()
    return nc


def _in_maps(output, target):
    x = np.asarray(output, dtype=np.float32)
    y = np.asarray(target, dtype=np.float32)
    xq = x.astype(_F8)          # [B, D] fp8
    yq = y.astype(_F8)
    xqf = xq.astype(np.float32)
    yqf = yq.astype(np.float32)
    xx = np.einsum("ij,ij->i", xqf, xqf)             # [B] f32
    yy = np.einsum("ij,ij->i", yqf, yqf)             # [B] f32
    m2yqT = np.ascontiguousarray((-2.0 * yqf).T.astype(_F8))  # [D, B], exact
    yy4 = (yy / 4.0).astype(_F8)                     # [B] fp8
    eye = np.eye(P, dtype=np.float32)
    four = np.float32(4.0).astype(_F8)

    maps = []
    for c in range(C):
        rows = slice(c * M, (c + 1) * M)
        w8 = np.zeros((P, NM, 2, P), _F8)
        w8[:, :, 0, :] = xq[rows].T.reshape(P, NM, P)
        w8[0, :, 1, :] = four
        maps.append(
            {
                "w8": w8,
                "rhs8m": np.ascontiguousarray(np.roll(m2yqT, -c * M, axis=1)),
                "rhs8yy": np.ascontiguousarray(np.roll(yy4, -c * M)[None, :]),
                "xxT": np.ascontiguousarray(xx[rows].reshape(NM, P).T),
                "eye": eye,
            }
        )
    return maps


def kernel(output, target):
    global _nc, LAST_RESULT
    if _nc is None:
        _nc = _build()
    maps = _in_maps(output, target)

    from concourse.bass_utils import run_bass_kernel_spmd

    res = run_bass_kernel_spmd(
        _nc, maps, core_ids=list(range(C)), trace=TRACE
    )
    LAST_RESULT = res

    tot = np.float64(0.0)
    dg = np.float64(0.0)
    for r in res.results:
        o = np.asarray(r["out"], dtype=np.float64)
        tot += o[:, : NM * NG].sum()
        dg += o[:, NM * NG : NM * NG + NM].sum()
    loss = (tot - 2.0 * dg) / B * 0.1
    return np.float32(loss)


# revision 9
# speedup vs baseline: 1.1002x; 1.0186x over previous
"""Pairwise-distance loss kernel for Trainium2 (8 NeuronCores, SPMD).

loss = (total_sum - 2*diag_sum) / B * 0.1  over the [B, B] matrix
d[i, n] = ||output[i] - target[n]||_2,  B=8192, D=128.

Sharding: core c owns rows [c*1024, (c+1)*1024) of `output` and all 8192
`target` columns. Per 128-row block m and 2048-col group g (4 PSUM banks):
  PSUM[i, n] = yy[n] - 2 * x_i . y_n   via ONE fp8 DoubleRow matmul per
  512-col bank: K=256 packed as [128, 2]: plane 0 carries the 128 features
  (weights = xq, moving = -2*yq), plane 1 carries the rank-1 yy term
  (weights row0 = 4.0, moving row0 = yy/4 in fp8).
  ACT: d = sqrt(PSUM + xx_i) with per-partition bias, accum_out = row sums.
Diagonal: each core's target columns are rotated by -c*1024 on the host so
the diagonal of row-block m always sits at local columns [m*128, (m+1)*128)
of group 0 — extracted with an eye-mask multiply + reduce on the vector
engine. The 8 cores' partial sums are combined on the host.
"""

import numpy as np
import ml_dtypes
from contextlib import ExitStack

B = 8192
D = 128
C = 8          # cores
M = B // C     # 1024 rows per core
P = 128        # partitions / row-block height
NM = M // P    # 8 row-blocks per core
GW = 2048      # ACT group width (4 PSUM banks)
NG = B // GW   # 4 groups
TS = 512       # matmul moving-dim tile (1 PSUM bank of f32)
NS = GW // TS  # 4 slices per group

_F8 = np.dtype(ml_dtypes.float8_e4m3)

# test.py can flip these before calling kernel() to capture an NTFF profile.
TRACE = False
LAST_RESULT = None

_nc = None


def _build():
    from concourse import bacc, bass, tile, mybir

    f32 = mybir.dt.float32
    fp8 = mybir.dt.float8e4
    nc = bacc.Bacc("TRN2", target_bir_lowering=False, debug=False)

    w8 = nc.dram_tensor("w8", [P, NM, 2, P], fp8, kind="ExternalInput").ap()
    rhs8m = nc.dram_tensor("rhs8m", [P, B], fp8, kind="ExternalInput").ap()
    rhs8yy = nc.dram_tensor("rhs8yy", [1, B], fp8, kind="ExternalInput").ap()
    xxT = nc.dram_tensor("xxT", [P, NM], f32, kind="ExternalInput").ap()
    eye = nc.dram_tensor("eye", [P, P], f32, kind="ExternalInput").ap()
    out = nc.dram_tensor("out", [P, NM * NG + NM], f32, kind="ExternalOutput").ap()

    with tile.TileContext(nc) as tc, ExitStack() as ctx:
        const = ctx.enter_context(tc.tile_pool(name="const", bufs=1))
        psum = ctx.enter_context(
            tc.tile_pool(name="psum", bufs=2, space=bass.MemorySpace.PSUM)
        )
        dpool = ctx.enter_context(tc.tile_pool(name="dtile", bufs=3))

        xx_s = const.tile([P, NM], f32)
        w8_s = const.tile([P, NM, 2, P], fp8)
        eye_s = const.tile([P, P], f32)
        rhs_s = const.tile([P, 2, B], fp8)
        # plane 1 rows 1.. are zeroed on idle engines (their products hit
        # zero weights, but must not be NaN); row 0 = yy/4
        plane1_u32 = rhs_s[:, 1, :].bitcast(mybir.dt.uint32)  # [P, B//4]
        nc.vector.memset(plane1_u32[:, 0 : B // 8], 0)
        nc.gpsimd.memset(plane1_u32[:, B // 8 : B // 4], 0)
        # DMA issue order favors what the first groups need: w8 + chunk 0
        # (split for queue parallelism), then the rest. yy goes on the
        # gpsimd SWDGE ring so it doesn't queue behind the bulk chunks.
        nc.sync.dma_start(w8_s[:], w8[:])
        nc.sync.dma_start(rhs_s[:, 0, 0 : GW // 2], rhs8m[:, 0 : GW // 2])
        nc.sync.dma_start(rhs_s[:, 0, GW // 2 : GW], rhs8m[:, GW // 2 : GW])
        nc.gpsimd.dma_start(rhs_s[0:1, 1, :], rhs8yy[:, :])
        nc.gpsimd.dma_start(xx_s[:], xxT[:])
        for g in range(1, NG):
            h = GW // 2
            nc.sync.dma_start(
                rhs_s[:, 0, g * GW : g * GW + h], rhs8m[:, g * GW : g * GW + h]
            )
            nc.sync.dma_start(
                rhs_s[:, 0, g * GW + h : (g + 1) * GW],
                rhs8m[:, g * GW + h : (g + 1) * GW],
            )
        nc.gpsimd.dma_start(eye_s[:], eye[:])

        accT = const.tile([P, NM * NG], f32)
        accD = const.tile([P, NM], f32)
        scr = const.tile([P, P], f32)

        for m in range(NM):
            for g in range(NG):
                pt = psum.tile([P, GW], f32)
                for s in range(NS):
                    n0 = g * GW + s * TS
                    nc.tensor.matmul(
                        pt[:, s * TS : (s + 1) * TS],
                        w8_s[:, m],
                        rhs_s[:, :, n0 : n0 + TS],
                        start=True,
                        stop=True,
                        perf_mode=mybir.MatmulPerfMode.DoubleRow,
                    )
                dt_ = dpool.tile([P, GW], f32)
                nc.scalar.activation(
                    dt_[:],
                    pt[:],
                    mybir.ActivationFunctionType.Sqrt,
                    bias=xx_s[:, m : m + 1],
                    scale=1.0,
                    accum_out=accT[:, m * NG + g : m * NG + g + 1],
                )
                if g == 0:
                    # diagonal of this row-block lives at local cols
                    # [m*128, (m+1)*128) thanks to the host-side rotation
                    # (tensor_tensor_reduce is avoided: it wedges the HW)
                    nc.vector.tensor_tensor(
                        out=scr[:],
                        in0=dt_[:, m * P : (m + 1) * P],
                        in1=eye_s[:],
                        op=mybir.AluOpType.mult,
                    )
                    nc.vector.reduce_sum(
                        accD[:, m : m + 1], scr[:], axis=mybir.AxisListType.X
                    )

        nc.sync.dma_start(out[:, 0 : NM * NG], accT[:])
        nc.sync.dma_start(out[:, NM * NG : NM * NG + NM], accD[:])

    nc.compile()
    return nc


def _in_maps(output, target):
    x = np.asarray(output, dtype=np.float32)
    y = np.asarray(target, dtype=np.float32)
    xq = x.astype(_F8)          # [B, D] fp8
    yq = y.astype(_F8)
    xqf = xq.astype(np.float32)
    yqf = yq.astype(np.float32)
    xx = np.einsum("ij,ij->i", xqf, xqf)             # [B] f32
    yy = np.einsum("ij,ij->i", yqf, yqf)             # [B] f32
    m2yqT = np.ascontiguousarray((-2.0 * yqf).T.astype(_F8))  # [D, B], exact
    yy4 = (yy / 4.0).astype(_F8)                     # [B] fp8
    eye = np.eye(P, dtype=np.float32)
    four = np.float32(4.0).astype(_F8)

    maps = []
    for c in range(C):
        rows = slice(c * M, (c + 1) * M)
        w8 = np.zeros((P, NM, 2, P), _F8)
        w8[:, :, 0, :] = xq[rows].T.reshape(P, NM, P)
        w8[0, :, 1, :] = four
        maps.append(
            {
                "w8": w8,
                "rhs8m": np.ascontiguousarray(np.roll(m2yqT, -c * M, axis=1)),
                "rhs8yy": np.ascontiguousarray(np.roll(yy4, -c * M)[None, :]),
                "xxT": np.ascontiguousarray(xx[rows].reshape(NM, P).T),
                "eye": eye,
            }
        )
    return maps


def kernel(output, target):
    global _nc, LAST_RESULT
    if _nc is None:
        _nc = _build()
    maps = _in_maps(output, target)

    from concourse.bass_utils import run_bass_kernel_spmd

    res = run_bass_kernel_spmd(
        _nc, maps, core_ids=list(range(C)), trace=TRACE
    )
    LAST_RESULT = res

    tot = np.float64(0.0)
    dg = np.float64(0.0)
    for r in res.results:
        o = np.asarray(r["out"], dtype=np.float64)
        tot += o[:, : NM * NG].sum()
        dg += o[:, NM * NG : NM * NG + NM].sum()
    loss = (tot - 2.0 * dg) / B * 0.1
    return np.float32(loss)
